# revision 8
# baseline (speedup 1.0000x reference)
"""Multi-head attention (B=2, S=2048, D=1024, H=16, causal, unscaled scores)
on 8 Trainium2 NeuronCores.

Sharding: 2 batches x 4 head-groups (4 heads each). Core c handles batch
c//4, heads 4*(c%4) .. 4*(c%4)+3. Each core computes its group's QKV
projections, causal attention, and a partial output projection
(row-slice of wo); the host sums the 4 partials per batch (the
all-reduce) and adds the bias terms.

Precision: Q/K path (xq, xk, wq, wk, qht, kht, scores) stays float32r
(bf16-pair fp32, ~1e-4) so the exp() arguments are accurate; the V path
(xv, wv, vh, exp-probabilities, attention output, wo, final output) is
bf16 — measured end-to-end rel err ~4e-3 against fp32, well inside the
2e-2 gate, and it halves HBM traffic + DVE cost on that side.

Schedule (v2, PE-density-first):
  - 16 warmup outer-product matmuls at t~0 keep the PE HAM activity
    window busy so the real stream starts at 2.4 GHz.
  - x/w loads are one strided DMA per (tensor, m-slice); xq/xk go on
    the sync HWDGE ring, xv/weights/outputs on the scalar ring so big
    Q/K streams never head-of-line-block V loads or output stores.
  - projection work is emitted in small chunks INSIDE the attention
    unit loop (attention i-slice IS overlaps projections m=IS+1), so
    the PE never stalls on the x DMA stream.
  - output projection runs one i-slice behind attention; softmax
    normalization is deferred: denominator rows (accumulated by the
    ones-column of VH during attnV) are copied to SBUF right away
    (freeing the U PSUM banks), reciprocals run batched [2,512] per
    head-pair on DVE off the critical path, and a K=2 selector matmul
    broadcasts both reciprocal rows into a [128,512] bank for one
    full-width in-place multiply of ct.
  - bias terms bv/bo are folded in exactly on the host
    (C = U/colsum + 1*bv since softmax rows sum to 1).
"""

import numpy as np

D = 1024
S = 2048
NH = 16
DH = 64
B = 2
G = 4            # head-groups = cores per batch
HG = NH // G     # 4 heads per group
GD = HG * DH     # 256 columns per group
KT = D // 128    # 8 k-tiles
MS = S // 512    # 4 m-slices
JT = S // 128    # 16 j-tiles
IST = S // 512   # 4 i-slices

_cached = None

_SEL = np.zeros((2, 128), np.float32)
_SEL[0, 0:64] = 1.0
_SEL[1, 64:128] = 1.0


def _build():
    from concourse import bacc
    import concourse.mybir as mybir
    import concourse.tile as tile

    f32 = mybir.dt.float32
    f32r = mybir.dt.float32r
    f16 = mybir.dt.float16
    bf16 = mybir.dt.bfloat16
    Act = mybir.ActivationFunctionType
    Alu = mybir.AluOpType

    nc = bacc.Bacc(None, target_bir_lowering=False)
    xq = nc.dram_tensor("xq", [D, S], f16, kind="ExternalInput")
    xk = nc.dram_tensor("xk", [D, S], f16, kind="ExternalInput")
    xv = nc.dram_tensor("xv", [D, S], bf16, kind="ExternalInput")
    wqg = nc.dram_tensor("wqg", [D, GD], f16, kind="ExternalInput")
    wkg = nc.dram_tensor("wkg", [D, GD], f16, kind="ExternalInput")
    wvg = nc.dram_tensor("wvg", [D, GD], bf16, kind="ExternalInput")
    wog = nc.dram_tensor("wog", [GD, D], bf16, kind="ExternalInput")
    bqg = nc.dram_tensor("bqg", [2, 128, 1], f32, kind="ExternalInput")
    bkg = nc.dram_tensor("bkg", [2, 128, 1], f32, kind="ExternalInput")
    selg = nc.dram_tensor("selg", [2, 128], f32r, kind="ExternalInput")
    outp = nc.dram_tensor("outp", [S, D], bf16, kind="ExternalOutput")

    with tile.TileContext(nc) as tc:
        with (
            tc.tile_pool(name="wpool", bufs=1) as wpool,
            tc.tile_pool(name="xqk", bufs=2) as xqk,
            tc.tile_pool(name="xvs", bufs=2) as xvs,
            tc.tile_pool(name="big", bufs=1) as big,
            tc.tile_pool(name="ppool", bufs=8) as ppool,
            tc.tile_pool(name="small", bufs=4) as small,
            tc.tile_pool(name="osb", bufs=4) as osb,
            tc.tile_pool(name="ps", bufs=2, space="PSUM") as ps,
            tc.tile_pool(name="po", bufs=2, space="PSUM") as po,
            tc.tile_pool(name="psU", bufs=2, space="PSUM") as psU,
        ):
            # ---- resident weights / constants ----
            wq_t = wpool.tile([128, KT, GD], f16, tag="wq")
            wk_t = wpool.tile([128, KT, GD], f16, tag="wk")
            wv_t = wpool.tile([128, KT, GD], bf16, tag="wv")
            wo_t = wpool.tile([128, 2, D], bf16, tag="wo")
            bq_t = wpool.tile([128, 2, 1], f32, tag="bq")
            bk_t = wpool.tile([128, 2, 1], f32, tag="bk")
            sel0 = wpool.tile([1, 128], f32r, tag="sel0")
            sel1 = wpool.tile([1, 128], f32r, tag="sel1")
            warm_sink = wpool.tile([1, 16], f32, tag="wsink")

            # selector rows first (tiny) so warmup matmuls start ~t=0
            nc.sync.dma_start(out=sel0, in_=selg[0:1, :])
            nc.sync.dma_start(out=sel1, in_=selg[1:2, :])

            # ---- PE warmup: ~3.5us of junk outer products so the HAM
            # un-throttles before the first projection matmul ----
            wpsum = po.tile([128, 128], f32, tag="po", name="warm")
            for i in range(32):
                nc.tensor.matmul(
                    wpsum,
                    sel0,
                    sel0,
                    start=(i == 0),
                    stop=(i == 31),
                )
            nc.vector.tensor_copy(warm_sink, wpsum[0:1, 0:16])

            # ---- input streams ----
            # sync ring: wq, xq(m0), wk, xk(m0), then xq/xk m1..3
            # scalar ring: bq, bk, wv, xv(m0), wo, xv m1..3 (+ outputs later)
            nc.sync.dma_start(out=wq_t[:, 0:2, :], in_=xq_like_w(wqg)[:, 0:2, :])
            nc.scalar.dma_start(out=bq_t, in_=bqg[:].rearrange("t p o -> p t o"))
            nc.scalar.dma_start(out=bk_t, in_=bkg[:].rearrange("t p o -> p t o"))

            xq_ts, xk_ts, xv_ts = [], [], []
            xq_r = xq[:].rearrange("(kt p) s -> p kt s", p=128)
            xk_r = xk[:].rearrange("(kt p) s -> p kt s", p=128)
            xv_r = xv[:].rearrange("(kt p) s -> p kt s", p=128)

            def load_m(m):
                ms = slice(m * 512, (m + 1) * 512)
                xqt = xqk.tile([128, KT, 512], f16, tag="xq", name="xqt")
                xkt = xqk.tile([128, KT, 512], f16, tag="xk", name="xkt")
                xvt = xvs.tile([128, KT, 512], bf16, tag="xv", name="xvt")
                if m == 0:
                    # split m0 so the first Q matmuls can start sooner
                    nc.sync.dma_start(out=xqt[:, 0:4, :], in_=xq_r[:, 0:4, ms])
                    nc.sync.dma_start(out=xqt[:, 4:KT, :], in_=xq_r[:, 4:KT, ms])
                else:
                    nc.sync.dma_start(out=xqt, in_=xq_r[:, :, ms])
                if m == 0:
                    nc.sync.dma_start(out=wq_t[:, 2:KT, :], in_=xq_like_w(wqg)[:, 2:KT, :])
                    nc.sync.dma_start(out=wk_t, in_=xq_like_w(wkg))
                nc.sync.dma_start(out=xkt, in_=xk_r[:, :, ms])
                if m == 0:
                    nc.sync.dma_start(out=wv_t, in_=xq_like_w(wvg))
                nc.sync.dma_start(out=xvt, in_=xv_r[:, :, ms])
                if m == 0:
                    nc.sync.dma_start(
                        out=wo_t, in_=wog[:].rearrange("(t p) n -> p t n", p=128)
                    )
                xq_ts.append(xqt)
                xk_ts.append(xkt)
                xv_ts.append(xvt)

            for m in range(MS):
                load_m(m)

            # ---- persistent activations ----
            qht = big.tile([128, 2, S], f16, tag="qht")
            kht = big.tile([128, 2, S], f16, tag="kht")
            vh = big.tile([128, JT, HG, DH + 1], bf16, tag="vh")
            ct = big.tile([128, 2, S], bf16, tag="ct")
            vh_ones_stage = wpool.tile([128, JT, HG, 1], f32, tag="vh_ones_st")
            nc.vector.memset(vh_ones_stage, 1.0)
            nc.scalar.activation(
                out=vh[:, :, :, DH : DH + 1], in_=vh_ones_stage, func=Act.Copy
            )

            # ---- projection chunk emitters (consumed as fillers) ----
            def qk_chunks(m):
                """QK projection chunk callables for m-slice m."""
                ms = slice(m * 512, (m + 1) * 512)

                def qk_chunk(xts, w_t, b_t, dst, n):
                    def emit():
                        psum = po.tile([128, 512], f32, tag="po")
                        for kk in range(KT):
                            nc.tensor.matmul(
                                psum,
                                w_t[:, kk, n * 128 : (n + 1) * 128],
                                xts[:, kk, :],
                                start=(kk == 0),
                                stop=(kk == KT - 1),
                            )
                        nc.vector.tensor_scalar_add(dst[:, n, ms], psum, b_t[:, n, :])

                    return emit

                out = []
                for n in range(2):
                    out.append(qk_chunk(xq_ts[m], wq_t, bq_t, qht, n))
                for n in range(2):
                    out.append(qk_chunk(xk_ts[m], wk_t, bk_t, kht, n))
                return out

            def v_chunks(m):
                def v_chunk(jj):
                    def emit():
                        j = m * 4 + jj
                        psum = po.tile([128, GD], f32, tag="po", name="vps")
                        for kk in range(KT):
                            nc.tensor.matmul(
                                psum,
                                xv_ts[m][:, kk, jj * 128 : (jj + 1) * 128],
                                wv_t[:, kk, :],
                                start=(kk == 0),
                                stop=(kk == KT - 1),
                            )
                        nc.vector.tensor_copy(
                            vh[:, j, :, 0:DH],
                            psum[:].rearrange("p (h d) -> p h d", h=HG),
                        )

                    return emit

                return [v_chunk(jj) for jj in range(4)]

            # ---- attention ----
            recips = {}

            def emit_attention_pair(IS, hp, filler=None):
                """Scores+exp+attnV pipeline for head-pair hp of i-slice IS.
                filler() is called between units to interleave other PE work."""
                i0 = IS * 512
                n_j = (IS + 1) * 4
                nt = hp
                u_psums = [
                    psU.tile([128, 512], f32, tag="u", name=f"u{e}")
                    for e in range(2)
                ]
                n_full = n_j - 4
                units = []
                for Jg in range(n_full // 2):
                    units.append(("full", Jg))
                for J in range(n_full, n_j):
                    units.append(("diag", J))
                pts = {}
                s_psums = {}

                def emit_scores(u):
                    kind, idx = u
                    if kind == "full":
                        for e in range(2):
                            lo = 64 * e
                            s_psum = ps.tile([128, 2, 512], f32, tag="ps")
                            for half in range(2):
                                J = 2 * idx + half
                                nc.tensor.matmul(
                                    s_psum[:, half, :],
                                    kht[lo : lo + DH, nt, J * 128 : (J + 1) * 128],
                                    qht[lo : lo + DH, nt, i0 : i0 + 512],
                                    start=True,
                                    stop=True,
                                )
                            s_psums[(e, u)] = s_psum
                    else:
                        J = idx
                        r = J * 128 - i0
                        s_psum = ps.tile([128, 2, 512], f32, tag="ps", name="sd")
                        for e in range(2):
                            lo = 64 * e
                            nc.tensor.matmul(
                                s_psum[:, e, 0 : 512 - r],
                                kht[lo : lo + DH, nt, J * 128 : (J + 1) * 128],
                                qht[lo : lo + DH, nt, i0 + r : i0 + 512],
                                start=True,
                                stop=True,
                            )
                        s_psums[(0, u)] = s_psum

                def emit_exp_mask(u):
                    kind, idx = u
                    if kind == "full":
                        for e in range(2):
                            pt = ppool.tile([128, 2, 512], bf16, tag="pt")
                            nc.scalar.activation(
                                out=pt, in_=s_psums[(e, u)], func=Act.Exp
                            )
                            pts[(e, u)] = pt
                    else:
                        r = idx * 128 - i0
                        w = 512 - r
                        pt = ppool.tile([128, 2, 512], bf16, tag="pt", name="ptd")
                        nc.scalar.activation(
                            out=pt[:, :, 0:w],
                            in_=s_psums[(0, u)][:, :, 0:w],
                            func=Act.Exp,
                        )
                        nc.gpsimd.affine_select(
                            out=pt[:, :, 0:w],
                            in_=pt[:, :, 0:w],
                            compare_op=Alu.is_ge,
                            fill=0.0,
                            base=0,
                            pattern=[[0, 2], [1, w]],
                            channel_multiplier=-1,
                        )
                        pts[(0, u)] = pt

                def emit_attnv(u):
                    kind, idx = u
                    for e in range(2):
                        if kind == "full":
                            for half in range(2):
                                J = 2 * idx + half
                                nc.tensor.matmul(
                                    u_psums[e][0 : DH + 1, :],
                                    vh[:, J, 2 * hp + e, :],
                                    pts[(e, u)][:, half, :],
                                    start=(J == 0),
                                    stop=False,
                                )
                        else:
                            J = idx
                            r = J * 128 - i0
                            nc.tensor.matmul(
                                u_psums[e][0 : DH + 1, r:512],
                                vh[:, J, 2 * hp + e, :],
                                pts[(0, u)][:, e, 0 : 512 - r],
                                start=(J == 0),
                                stop=(J == n_j - 1),
                            )

                emit_scores(units[0])
                emit_exp_mask(units[0])
                for ui in range(1, len(units)):
                    emit_scores(units[ui])
                    emit_exp_mask(units[ui])
                    emit_attnv(units[ui - 1])
                    if filler is not None:
                        filler()
                emit_attnv(units[-1])

                # evacuate U banks: ct rows out, fast-approx reciprocal of the
                # denominator row straight from PSUM (~5x cheaper than the
                # iterative DVE reciprocal; denominators are strictly positive
                # finite so the approx edge cases can't occur), then a trivial
                # f32 -> f32r convert-copy for the selector matmul.
                for e in range(2):
                    lo = 64 * e
                    nc.vector.tensor_copy(
                        ct[lo : lo + DH, nt, i0 : i0 + 512], u_psums[e][0:DH, :]
                    )
                    # NOTE: reciprocal_approx_fast's custom ucode ignores the
                    # partition offset on PSUM reads (HW-probed), so stage the
                    # denominator row to SBUF partition 0 first.  The staging
                    # copy also releases the U PSUM bank quickly.
                    rden = small.tile([1, 512], f32, tag="rden", name=f"rd{e}")
                    rcf = small.tile([1, 512], f32, tag="rcf", name=f"rcf{e}")
                    rc = small.tile([1, 512], f32r, tag="rc", name=f"rc{e}")
                    nc.vector.tensor_copy(rden, u_psums[e][DH : DH + 1, :])
                    nc.vector.reciprocal_approx_fast(out=rcf, in_=rden)
                    nc.vector.tensor_copy(rc, rcf)
                    recips[(IS, hp, e)] = rc

            def emit_normalize(IS):
                i0 = IS * 512
                for hp in range(HG // 2):
                    bc_psum = po.tile([128, 512], f32, tag="po", name="bcp")
                    for e, sel in ((0, sel0), (1, sel1)):
                        nc.tensor.matmul(
                            bc_psum,
                            sel,
                            recips[(IS, hp, e)],
                            start=(e == 0),
                            stop=(e == 1),
                        )
                    nc.vector.tensor_mul(
                        ct[:, hp, i0 : i0 + 512],
                        bc_psum,
                        ct[:, hp, i0 : i0 + 512],
                    )

            def outproj_its(IS):
                def mk(it):
                    def emit():
                        r0 = IS * 512 + it * 128
                        out_sb = osb.tile([128, D], bf16, tag="out")
                        for nn in range(2):
                            o_psum = po.tile([128, 512], f32, tag="po")
                            for t in range(2):
                                nc.tensor.matmul(
                                    o_psum,
                                    ct[:, t, r0 : r0 + 128],
                                    wo_t[:, t, nn * 512 : (nn + 1) * 512],
                                    start=(t == 0),
                                    stop=(t == 1),
                                )
                            nc.vector.tensor_copy(
                                out_sb[:, nn * 512 : (nn + 1) * 512], o_psum
                            )
                        nc.gpsimd.dma_start(out=outp[r0 : r0 + 128, :], in_=out_sb)

                    return emit

                return [mk(it) for it in range(4)]

            # ---- main schedule ----
            # Per-block PE/ACT balance: exp (ACT) grows with the i-slice
            # (causal triangle) while projection fillers shrink, so outproj
            # is deferred ~2 slices and m3's V chunks land in block 3 to
            # feed the PE there.  Normalize stays 1 slice behind attention.
            #   B0: att(0) + proj(m1)
            #   B1: att(1) + proj(m2)                 [norm(0) between pairs]
            #   B2: att(2) + proj(m3) QK + outproj(0) [norm(1)]
            #   B3: att(3) + proj(m3) V + outproj(1,2)[norm(2)]
            #   tail: norm(3) + outproj(3)
            for emit in qk_chunks(0) + v_chunks(0):
                emit()

            blocks = [
                (qk_chunks(1) + v_chunks(1), [], None),
                (qk_chunks(2) + v_chunks(2), [], 0),
                (qk_chunks(3) + outproj_its(0), [], 1),
                (v_chunks(3) + outproj_its(1), outproj_its(2), 2),
            ]
            for IS, (fill0, fill1, norm_is) in enumerate(blocks):
                pending = list(fill0)
                pending.reverse()

                def filler():
                    if pending:
                        pending.pop()()

                emit_attention_pair(IS, 0, filler=filler)
                while pending:
                    pending.pop()()
                if norm_is is not None:
                    emit_normalize(norm_is)
                pending = list(fill1)
                pending.reverse()
                emit_attention_pair(IS, 1, filler=filler)
                while pending:
                    pending.pop()()
            emit_normalize(IST - 1)
            for emit in outproj_its(IST - 1):
                emit()

    nc.compile()
    return nc


def xq_like_w(w):
    return w[:].rearrange("(kt p) n -> p kt n", p=128)


def _get_nc():
    global _cached
    if _cached is None:
        _cached = _build()
    return _cached


def _in_maps(q, k, v, wq, bq, wk, bk, wv, bv, wo, bo):
    import ml_dtypes

    bf = ml_dtypes.bfloat16
    maps = []
    for c in range(8):
        b, g = c // G, c % G
        cs = slice(g * GD, (g + 1) * GD)
        maps.append(
            {
                "xq": np.ascontiguousarray(q[b].T).astype(np.float16),
                "xk": np.ascontiguousarray(k[b].T).astype(np.float16),
                "xv": np.ascontiguousarray(v[b].T).astype(bf),
                "wqg": np.ascontiguousarray(wq[:, cs]).astype(np.float16),
                "wkg": np.ascontiguousarray(wk[:, cs]).astype(np.float16),
                "wvg": np.ascontiguousarray(wv[:, cs]).astype(bf),
                "wog": np.ascontiguousarray(wo[cs, :]).astype(bf),
                "bqg": np.ascontiguousarray(bq[cs]).reshape(2, 128, 1),
                "bkg": np.ascontiguousarray(bk[cs]).reshape(2, 128, 1),
                "selg": _SEL,
            }
        )
    return maps


def run(inputs, trace=False, trace_kwargs=None):
    from concourse.bass_utils import run_bass_kernel_spmd

    nc = _get_nc()
    maps = _in_maps(**inputs)
    res = run_bass_kernel_spmd(
        nc, maps, list(range(8)), trace=trace, **(trace_kwargs or {})
    )
    out = np.zeros((B, S, D), np.float32)
    for c in range(8):
        out[c // G] += res.results[c]["outp"].astype(np.float32)
    # exact bias fold: C = U/colsum + 1 (x) bv  =>  out += bv @ wo + bo
    out += inputs["bv"].astype(np.float32) @ inputs["wo"].astype(np.float32)
    out += inputs["bo"].astype(np.float32)
    return out.astype(np.float32), res


def kernel(**inputs) -> np.ndarray:
    out, _ = run(inputs)
    return out


# revision 10
# speedup vs baseline: 1.0044x; 1.0044x over previous
"""Multi-head attention (B=2, S=2048, D=1024, H=16, causal, unscaled scores)
on 8 Trainium2 NeuronCores.

Sharding: 2 batches x 4 head-groups (4 heads each). Core c handles batch
c//4, heads 4*(c%4) .. 4*(c%4)+3. Each core computes its group's QKV
projections, causal attention, and a partial output projection
(row-slice of wo); the host sums the 4 partials per batch (the
all-reduce) and adds the bias terms.

Precision: Q/K path (xq, xk, wq, wk, qht, kht, scores) stays float32r
(bf16-pair fp32, ~1e-4) so the exp() arguments are accurate; the V path
(xv, wv, vh, exp-probabilities, attention output, wo, final output) is
bf16 — measured end-to-end rel err ~4e-3 against fp32, well inside the
2e-2 gate, and it halves HBM traffic + DVE cost on that side.

Schedule (v2, PE-density-first):
  - 16 warmup outer-product matmuls at t~0 keep the PE HAM activity
    window busy so the real stream starts at 2.4 GHz.
  - x/w loads are one strided DMA per (tensor, m-slice); xq/xk go on
    the sync HWDGE ring, xv/weights/outputs on the scalar ring so big
    Q/K streams never head-of-line-block V loads or output stores.
  - projection work is emitted in small chunks INSIDE the attention
    unit loop (attention i-slice IS overlaps projections m=IS+1), so
    the PE never stalls on the x DMA stream.
  - output projection runs one i-slice behind attention; softmax
    normalization is deferred: denominator rows (accumulated by the
    ones-column of VH during attnV) are copied to SBUF right away
    (freeing the U PSUM banks), reciprocals run batched [2,512] per
    head-pair on DVE off the critical path, and a K=2 selector matmul
    broadcasts both reciprocal rows into a [128,512] bank for one
    full-width in-place multiply of ct.
  - bias terms bv/bo are folded in exactly on the host
    (C = U/colsum + 1*bv since softmax rows sum to 1).
"""

import numpy as np

D = 1024
S = 2048
NH = 16
DH = 64
B = 2
G = 4            # head-groups = cores per batch
HG = NH // G     # 4 heads per group
GD = HG * DH     # 256 columns per group
KT = D // 128    # 8 k-tiles
MS = S // 512    # 4 m-slices
JT = S // 128    # 16 j-tiles
IST = S // 512   # 4 i-slices

_cached = None

_SEL = np.zeros((2, 128), np.float32)
_SEL[0, 0:64] = 1.0
_SEL[1, 64:128] = 1.0


def _build():
    from concourse import bacc
    import concourse.mybir as mybir
    import concourse.tile as tile

    f32 = mybir.dt.float32
    f32r = mybir.dt.float32r
    f16 = mybir.dt.float16
    bf16 = mybir.dt.bfloat16
    Act = mybir.ActivationFunctionType
    Alu = mybir.AluOpType

    nc = bacc.Bacc(None, target_bir_lowering=False)
    xq = nc.dram_tensor("xq", [D, S], f16, kind="ExternalInput")
    xk = nc.dram_tensor("xk", [D, S], f16, kind="ExternalInput")
    xv = nc.dram_tensor("xv", [D, S], bf16, kind="ExternalInput")
    wqg = nc.dram_tensor("wqg", [D, GD], f16, kind="ExternalInput")
    wkg = nc.dram_tensor("wkg", [D, GD], f16, kind="ExternalInput")
    wvg = nc.dram_tensor("wvg", [D, GD], bf16, kind="ExternalInput")
    wog = nc.dram_tensor("wog", [GD, D], bf16, kind="ExternalInput")
    bqg = nc.dram_tensor("bqg", [2, 128, 1], f32, kind="ExternalInput")
    bkg = nc.dram_tensor("bkg", [2, 128, 1], f32, kind="ExternalInput")
    selg = nc.dram_tensor("selg", [2, 128], f32r, kind="ExternalInput")
    outp = nc.dram_tensor("outp", [S, D], bf16, kind="ExternalOutput")

    with tile.TileContext(nc) as tc:
        with (
            tc.tile_pool(name="wpool", bufs=1) as wpool,
            tc.tile_pool(name="xqk", bufs=2) as xqk,
            tc.tile_pool(name="xvs", bufs=2) as xvs,
            tc.tile_pool(name="big", bufs=1) as big,
            tc.tile_pool(name="ppool", bufs=8) as ppool,
            tc.tile_pool(name="small", bufs=4) as small,
            tc.tile_pool(name="osb", bufs=4) as osb,
            tc.tile_pool(name="ps", bufs=2, space="PSUM") as ps,
            tc.tile_pool(name="po", bufs=2, space="PSUM") as po,
            tc.tile_pool(name="psU", bufs=2, space="PSUM") as psU,
        ):
            # ---- resident weights / constants ----
            wq_t = wpool.tile([128, KT, GD], f16, tag="wq")
            wk_t = wpool.tile([128, KT, GD], f16, tag="wk")
            wv_t = wpool.tile([128, KT, GD], bf16, tag="wv")
            wo_t = wpool.tile([128, 2, D], bf16, tag="wo")
            bq_t = wpool.tile([128, 2, 1], f32, tag="bq")
            bk_t = wpool.tile([128, 2, 1], f32, tag="bk")
            sel0 = wpool.tile([1, 128], f32r, tag="sel0")
            sel1 = wpool.tile([1, 128], f32r, tag="sel1")
            warm_sink = wpool.tile([1, 16], f32, tag="wsink")

            # selector rows first (tiny) so warmup matmuls start ~t=0
            nc.sync.dma_start(out=sel0, in_=selg[0:1, :])
            nc.sync.dma_start(out=sel1, in_=selg[1:2, :])

            # ---- PE warmup: ~3.5us of junk outer products so the HAM
            # un-throttles before the first projection matmul ----
            wpsum = po.tile([128, 128], f32, tag="po", name="warm")
            for i in range(20):
                nc.tensor.matmul(
                    wpsum,
                    sel0,
                    sel0,
                    start=(i == 0),
                    stop=(i == 19),
                )
            nc.vector.tensor_copy(warm_sink, wpsum[0:1, 0:16])

            # ---- input streams ----
            # sync ring: wq, xq(m0), wk, xk(m0), then xq/xk m1..3
            # scalar ring: bq, bk, wv, xv(m0), wo, xv m1..3 (+ outputs later)
            nc.sync.dma_start(out=wq_t[:, 0:4, :], in_=xq_like_w(wqg)[:, 0:4, :])
            nc.scalar.dma_start(out=bq_t, in_=bqg[:].rearrange("t p o -> p t o"))
            nc.scalar.dma_start(out=bk_t, in_=bkg[:].rearrange("t p o -> p t o"))

            xq_ts, xk_ts, xv_ts = [], [], []
            xq_r = xq[:].rearrange("(kt p) s -> p kt s", p=128)
            xk_r = xk[:].rearrange("(kt p) s -> p kt s", p=128)
            xv_r = xv[:].rearrange("(kt p) s -> p kt s", p=128)

            def load_m(m):
                ms = slice(m * 512, (m + 1) * 512)
                xqt = xqk.tile([128, KT, 512], f16, tag="xq", name="xqt")
                xkt = xqk.tile([128, KT, 512], f16, tag="xk", name="xkt")
                xvt = xvs.tile([128, KT, 512], bf16, tag="xv", name="xvt")
                if m == 0:
                    # need-ordered halves so kk 0-3 matmuls start early
                    nc.sync.dma_start(out=xqt[:, 0:4, :], in_=xq_r[:, 0:4, ms])
                    nc.sync.dma_start(out=wq_t[:, 4:KT, :], in_=xq_like_w(wqg)[:, 4:KT, :])
                    nc.sync.dma_start(out=xqt[:, 4:KT, :], in_=xq_r[:, 4:KT, ms])
                    nc.sync.dma_start(out=wk_t[:, 0:4, :], in_=xq_like_w(wkg)[:, 0:4, :])
                else:
                    nc.sync.dma_start(out=xqt, in_=xq_r[:, :, ms])
                if m == 0:
                    nc.sync.dma_start(out=xkt[:, 0:4, :], in_=xk_r[:, 0:4, ms])
                    nc.sync.dma_start(out=wk_t[:, 4:KT, :], in_=xq_like_w(wkg)[:, 4:KT, :])
                    nc.sync.dma_start(out=xkt[:, 4:KT, :], in_=xk_r[:, 4:KT, ms])
                else:
                    nc.sync.dma_start(out=xkt, in_=xk_r[:, :, ms])
                if m == 0:
                    nc.sync.dma_start(out=wv_t, in_=xq_like_w(wvg))
                nc.sync.dma_start(out=xvt, in_=xv_r[:, :, ms])
                if m == 0:
                    nc.sync.dma_start(
                        out=wo_t, in_=wog[:].rearrange("(t p) n -> p t n", p=128)
                    )
                xq_ts.append(xqt)
                xk_ts.append(xkt)
                xv_ts.append(xvt)

            for m in range(MS):
                load_m(m)

            # ---- persistent activations ----
            qht = big.tile([128, 2, S], f16, tag="qht")
            kht = big.tile([128, 2, S], f16, tag="kht")
            vh = big.tile([128, JT, HG, DH + 1], bf16, tag="vh")
            ct = big.tile([128, 2, S], bf16, tag="ct")
            vh_ones_stage = wpool.tile([128, JT, HG, 1], f32, tag="vh_ones_st")
            nc.vector.memset(vh_ones_stage, 1.0)
            nc.scalar.activation(
                out=vh[:, :, :, DH : DH + 1], in_=vh_ones_stage, func=Act.Copy
            )

            # ---- projection chunk emitters (consumed as fillers) ----
            def qk_chunks(m):
                """QK projection chunk callables for m-slice m."""
                ms = slice(m * 512, (m + 1) * 512)
                qk_chunks.psums = {}

                def qk_chunk(xts, w_t, b_t, dst, n, kks):
                    def emit():
                        if kks[0] == 0:
                            psum = po.tile([128, 512], f32, tag="po", name="qkp")
                            qk_chunks.psums[(id(xts), n)] = psum
                        psum = qk_chunks.psums[(id(xts), n)]
                        for kk in kks:
                            nc.tensor.matmul(
                                psum,
                                w_t[:, kk, n * 128 : (n + 1) * 128],
                                xts[:, kk, :],
                                start=(kk == 0),
                                stop=(kk == KT - 1),
                            )
                        if kks[-1] == KT - 1:
                            nc.vector.tensor_scalar_add(
                                dst[:, n, ms], psum, b_t[:, n, :]
                            )

                    return emit

                out = []
                splits = (
                    [list(range(0, 4)), list(range(4, KT))]
                    if m == 0
                    else [list(range(KT))]
                )
                for n in range(2):
                    for kks in splits:
                        out.append(qk_chunk(xq_ts[m], wq_t, bq_t, qht, n, kks))
                for n in range(2):
                    for kks in splits:
                        out.append(qk_chunk(xk_ts[m], wk_t, bk_t, kht, n, kks))
                return out

            def v_chunks(m):
                def v_chunk(jj):
                    def emit():
                        j = m * 4 + jj
                        psum = po.tile([128, GD], f32, tag="po", name="vps")
                        for kk in range(KT):
                            nc.tensor.matmul(
                                psum,
                                xv_ts[m][:, kk, jj * 128 : (jj + 1) * 128],
                                wv_t[:, kk, :],
                                start=(kk == 0),
                                stop=(kk == KT - 1),
                            )
                        nc.vector.tensor_copy(
                            vh[:, j, :, 0:DH],
                            psum[:].rearrange("p (h d) -> p h d", h=HG),
                        )

                    return emit

                return [v_chunk(jj) for jj in range(4)]

            # ---- attention ----
            recips = {}

            def emit_attention_pair(IS, hp, filler=None):
                """Scores+exp+attnV pipeline for head-pair hp of i-slice IS.
                filler() is called between units to interleave other PE work."""
                i0 = IS * 512
                n_j = (IS + 1) * 4
                nt = hp
                u_psums = [
                    psU.tile([128, 512], f32, tag="u", name=f"u{e}")
                    for e in range(2)
                ]
                n_full = n_j - 4
                units = []
                for Jg in range(n_full // 2):
                    units.append(("full", Jg))
                for J in range(n_full, n_j):
                    units.append(("diag", J))
                pts = {}
                s_psums = {}

                def emit_scores(u):
                    kind, idx = u
                    if kind == "full":
                        for e in range(2):
                            lo = 64 * e
                            s_psum = ps.tile([128, 2, 512], f32, tag="ps")
                            for half in range(2):
                                J = 2 * idx + half
                                nc.tensor.matmul(
                                    s_psum[:, half, :],
                                    kht[lo : lo + DH, nt, J * 128 : (J + 1) * 128],
                                    qht[lo : lo + DH, nt, i0 : i0 + 512],
                                    start=True,
                                    stop=True,
                                )
                            s_psums[(e, u)] = s_psum
                    else:
                        J = idx
                        r = J * 128 - i0
                        s_psum = ps.tile([128, 2, 512], f32, tag="ps", name="sd")
                        for e in range(2):
                            lo = 64 * e
                            nc.tensor.matmul(
                                s_psum[:, e, 0 : 512 - r],
                                kht[lo : lo + DH, nt, J * 128 : (J + 1) * 128],
                                qht[lo : lo + DH, nt, i0 + r : i0 + 512],
                                start=True,
                                stop=True,
                            )
                        s_psums[(0, u)] = s_psum

                def emit_exp_mask(u):
                    kind, idx = u
                    if kind == "full":
                        for e in range(2):
                            pt = ppool.tile([128, 2, 512], bf16, tag="pt")
                            nc.scalar.activation(
                                out=pt, in_=s_psums[(e, u)], func=Act.Exp
                            )
                            pts[(e, u)] = pt
                    else:
                        r = idx * 128 - i0
                        w = 512 - r
                        pt = ppool.tile([128, 2, 512], bf16, tag="pt", name="ptd")
                        nc.scalar.activation(
                            out=pt[:, :, 0:w],
                            in_=s_psums[(0, u)][:, :, 0:w],
                            func=Act.Exp,
                        )
                        nc.gpsimd.affine_select(
                            out=pt[:, :, 0:w],
                            in_=pt[:, :, 0:w],
                            compare_op=Alu.is_ge,
                            fill=0.0,
                            base=0,
                            pattern=[[0, 2], [1, w]],
                            channel_multiplier=-1,
                        )
                        pts[(0, u)] = pt

                def emit_attnv(u):
                    kind, idx = u
                    for e in range(2):
                        if kind == "full":
                            for half in range(2):
                                J = 2 * idx + half
                                nc.tensor.matmul(
                                    u_psums[e][0 : DH + 1, :],
                                    vh[:, J, 2 * hp + e, :],
                                    pts[(e, u)][:, half, :],
                                    start=(J == 0),
                                    stop=False,
                                )
                        else:
                            J = idx
                            r = J * 128 - i0
                            nc.tensor.matmul(
                                u_psums[e][0 : DH + 1, r:512],
                                vh[:, J, 2 * hp + e, :],
                                pts[(0, u)][:, e, 0 : 512 - r],
                                start=(J == 0),
                                stop=(J == n_j - 1),
                            )

                emit_scores(units[0])
                emit_exp_mask(units[0])
                for ui in range(1, len(units)):
                    emit_scores(units[ui])
                    emit_exp_mask(units[ui])
                    emit_attnv(units[ui - 1])
                    if filler is not None:
                        filler()
                emit_attnv(units[-1])

                # evacuate U banks: ct rows out, fast-approx reciprocal of the
                # denominator row straight from PSUM (~5x cheaper than the
                # iterative DVE reciprocal; denominators are strictly positive
                # finite so the approx edge cases can't occur), then a trivial
                # f32 -> f32r convert-copy for the selector matmul.
                for e in range(2):
                    lo = 64 * e
                    nc.vector.tensor_copy(
                        ct[lo : lo + DH, nt, i0 : i0 + 512], u_psums[e][0:DH, :]
                    )
                    # NOTE: reciprocal_approx_fast's custom ucode ignores the
                    # partition offset on PSUM reads (HW-probed), so stage the
                    # denominator row to SBUF partition 0 first.  The staging
                    # copy also releases the U PSUM bank quickly.
                    rden = small.tile([1, 512], f32, tag="rden", name=f"rd{e}")
                    rcf = small.tile([1, 512], f32, tag="rcf", name=f"rcf{e}")
                    rc = small.tile([1, 512], f32r, tag="rc", name=f"rc{e}")
                    nc.vector.tensor_copy(rden, u_psums[e][DH : DH + 1, :])
                    nc.vector.reciprocal_approx_fast(out=rcf, in_=rden)
                    nc.vector.tensor_copy(rc, rcf)
                    recips[(IS, hp, e)] = rc

            def emit_normalize_hp(IS, hp):
                i0 = IS * 512
                bc_psum = po.tile([128, 512], f32, tag="po", name="bcp")
                for e, sel in ((0, sel0), (1, sel1)):
                    nc.tensor.matmul(
                        bc_psum,
                        sel,
                        recips[(IS, hp, e)],
                        start=(e == 0),
                        stop=(e == 1),
                    )
                nc.vector.tensor_mul(
                    ct[:, hp, i0 : i0 + 512],
                    bc_psum,
                    ct[:, hp, i0 : i0 + 512],
                )

            def emit_normalize(IS):
                for hp in range(HG // 2):
                    emit_normalize_hp(IS, hp)

            def outproj_its(IS):
                def mk(it):
                    def emit():
                        r0 = IS * 512 + it * 128
                        out_sb = osb.tile([128, D], bf16, tag="out")
                        for nn in range(2):
                            o_psum = po.tile([128, 512], f32, tag="po")
                            for t in range(2):
                                nc.tensor.matmul(
                                    o_psum,
                                    ct[:, t, r0 : r0 + 128],
                                    wo_t[:, t, nn * 512 : (nn + 1) * 512],
                                    start=(t == 0),
                                    stop=(t == 1),
                                )
                            nc.vector.tensor_copy(
                                out_sb[:, nn * 512 : (nn + 1) * 512], o_psum
                            )
                        nc.gpsimd.dma_start(out=outp[r0 : r0 + 128, :], in_=out_sb)

                    return emit

                return [mk(it) for it in range(4)]

            # ---- main schedule ----
            # Per-block PE/ACT balance: exp (ACT) grows with the i-slice
            # (causal triangle) while projection fillers shrink, so outproj
            # is deferred ~2 slices and m3's V chunks land in block 3 to
            # feed the PE there.  Normalize stays 1 slice behind attention.
            #   B0: att(0) + proj(m1)
            #   B1: att(1) + proj(m2)                 [norm(0) between pairs]
            #   B2: att(2) + proj(m3) QK + outproj(0) [norm(1)]
            #   B3: att(3) + proj(m3) V + outproj(1,2)[norm(2)]
            #   tail: norm(3) + outproj(3)
            for emit in qk_chunks(0) + v_chunks(0):
                emit()

            blocks = [
                (qk_chunks(1) + v_chunks(1), [], None),
                (qk_chunks(2) + v_chunks(2), [], 0),
                (qk_chunks(3) + outproj_its(0), [], 1),
                (v_chunks(3) + outproj_its(1), outproj_its(2), 2),
            ]
            for IS, (fill0, fill1, norm_is) in enumerate(blocks):
                pending = list(fill0)
                pending.reverse()

                def filler():
                    if pending:
                        pending.pop()()

                emit_attention_pair(IS, 0, filler=filler)
                while pending:
                    pending.pop()()
                if norm_is is not None:
                    emit_normalize(norm_is)
                pending = list(fill1)
                pending.reverse()
                if IS == IST - 1:
                    # normalize pair0 of the last slice early (its recips
                    # complete during pair1) so the tail only waits on pair1
                    pending.append(lambda: emit_normalize_hp(IST - 1, 0))
                emit_attention_pair(IS, 1, filler=filler)
                while pending:
                    pending.pop()()
            emit_normalize_hp(IST - 1, 1)
            for emit in outproj_its(IST - 1):
                emit()

    nc.compile()
    return nc


def xq_like_w(w):
    return w[:].rearrange("(kt p) n -> p kt n", p=128)


def _get_nc():
    global _cached
    if _cached is None:
        _cached = _build()
    return _cached


def _in_maps(q, k, v, wq, bq, wk, bk, wv, bv, wo, bo):
    import ml_dtypes

    bf = ml_dtypes.bfloat16
    maps = []
    for c in range(8):
        b, g = c // G, c % G
        cs = slice(g * GD, (g + 1) * GD)
        maps.append(
            {
                "xq": np.ascontiguousarray(q[b].T).astype(np.float16),
                "xk": np.ascontiguousarray(k[b].T).astype(np.float16),
                "xv": np.ascontiguousarray(v[b].T).astype(bf),
                "wqg": np.ascontiguousarray(wq[:, cs]).astype(np.float16),
                "wkg": np.ascontiguousarray(wk[:, cs]).astype(np.float16),
                "wvg": np.ascontiguousarray(wv[:, cs]).astype(bf),
                "wog": np.ascontiguousarray(wo[cs, :]).astype(bf),
                "bqg": np.ascontiguousarray(bq[cs]).reshape(2, 128, 1),
                "bkg": np.ascontiguousarray(bk[cs]).reshape(2, 128, 1),
                "selg": _SEL,
            }
        )
    return maps


def run(inputs, trace=False, trace_kwargs=None):
    from concourse.bass_utils import run_bass_kernel_spmd

    nc = _get_nc()
    maps = _in_maps(**inputs)
    res = run_bass_kernel_spmd(
        nc, maps, list(range(8)), trace=trace, **(trace_kwargs or {})
    )
    out = np.zeros((B, S, D), np.float32)
    for c in range(8):
        out[c // G] += res.results[c]["outp"].astype(np.float32)
    # exact bias fold: C = U/colsum + 1 (x) bv  =>  out += bv @ wo + bo
    out += inputs["bv"].astype(np.float32) @ inputs["wo"].astype(np.float32)
    out += inputs["bo"].astype(np.float32)
    return out.astype(np.float32), res


def kernel(**inputs) -> np.ndarray:
    out, _ = run(inputs)
    return out


# revision 13
# speedup vs baseline: 1.0226x; 1.0181x over previous
"""Multi-head attention (B=2, S=2048, D=1024, H=16, causal, unscaled scores)
on 8 Trainium2 NeuronCores.

Sharding: 2 batches x 4 head-groups (4 heads each). Core c handles batch
c//4, heads 4*(c%4) .. 4*(c%4)+3. Each core computes its group's QKV
projections, causal attention, and a partial output projection
(row-slice of wo); the host sums the 4 partials per batch (the
all-reduce) and adds the bias terms.

Precision: Q/K path (xq, xk, wq, wk, qht, kht) is fp16 (10-bit
mantissa; scores accumulate in fp32 PSUM so exp() arguments stay
accurate); the V path (xv, wv, vh, exp-probabilities, attention
output, wo, final output) is bf16.  Measured end-to-end rel err
~4.7e-3 against fp32 — well inside the 2e-2 gate — while halving HBM
traffic (14.7MB read/core) and DVE cost.

Schedule (PE-density-first — the kernel is TensorE-bound; the HAM
clock gate halves the PE clock after ~3.4us of idleness, so the
emission order keeps the PE stream dense):
  - 20 warmup outer-product matmuls at t~0 hold the HAM activity
    window busy during the DMA prologue so real matmuls start at
    2.4 GHz; m0 loads are split/need-ordered and the first QK
    projection chunks run on half the contraction while the rest
    streams in.
  - all input loads ride the sync HWDGE ring in need-order as one
    strided DMA per (tensor, m-slice); output stores ride the gpsimd
    SWDGE ring so they never contend with input streaming or the ACT
    engine (which is ~80% busy with exp in the late i-slices).
  - projection work is emitted in small chunks INSIDE the attention
    unit loop, balanced per block against the causal-triangle exp
    load: att(IS) overlaps proj(m=IS+1); outproj is deferred ~2
    slices and m3's V chunks land in the last block to feed the PE
    where exp dominates.
  - softmax normalization is deferred one slice: the denominator row
    (accumulated free by the ones-column of VH during attnV) is
    staged to SBUF partition 0 (reciprocal_approx_fast's custom ucode
    ignores partition offsets on PSUM reads — HW-probed), inverted
    with the ~5x-faster approx reciprocal, converted f32->f32r, and a
    K=1 selector matmul pair broadcasts both rows into a [128,512]
    bank for one full-width in-place multiply of ct.  The final
    pair's chain is split across DVE and ACT with an fp32 selector
    matmul to shorten the kernel tail.
  - bias terms bv/bo are folded in exactly on the host
    (C = U/colsum + 1*bv since softmax rows sum to 1).
"""

import numpy as np

D = 1024
S = 2048
NH = 16
DH = 64
B = 2
G = 4            # head-groups = cores per batch
HG = NH // G     # 4 heads per group
GD = HG * DH     # 256 columns per group
KT = D // 128    # 8 k-tiles
MS = S // 512    # 4 m-slices
JT = S // 128    # 16 j-tiles
IST = S // 512   # 4 i-slices

_cached = None

_SEL = np.zeros((2, 128), np.float32)
_SEL[0, 0:64] = 1.0
_SEL[1, 64:128] = 1.0


def _build():
    from concourse import bacc
    import concourse.mybir as mybir
    import concourse.tile as tile

    f32 = mybir.dt.float32
    f32r = mybir.dt.float32r
    f16 = mybir.dt.float16
    bf16 = mybir.dt.bfloat16
    Act = mybir.ActivationFunctionType
    Alu = mybir.AluOpType

    nc = bacc.Bacc(None, target_bir_lowering=False)
    xq = nc.dram_tensor("xq", [D, S], f16, kind="ExternalInput")
    xk = nc.dram_tensor("xk", [D, S], f16, kind="ExternalInput")
    xv = nc.dram_tensor("xv", [D, S], bf16, kind="ExternalInput")
    wqg = nc.dram_tensor("wqg", [D, GD], f16, kind="ExternalInput")
    wkg = nc.dram_tensor("wkg", [D, GD], f16, kind="ExternalInput")
    wvg = nc.dram_tensor("wvg", [D, GD], bf16, kind="ExternalInput")
    wog = nc.dram_tensor("wog", [GD, D], bf16, kind="ExternalInput")
    bqg = nc.dram_tensor("bqg", [2, 128, 1], f32, kind="ExternalInput")
    bkg = nc.dram_tensor("bkg", [2, 128, 1], f32, kind="ExternalInput")
    selg = nc.dram_tensor("selg", [2, 128], f32r, kind="ExternalInput")
    selg2 = nc.dram_tensor("selg2", [2, 128], f32, kind="ExternalInput")
    outp = nc.dram_tensor("outp", [S, D], bf16, kind="ExternalOutput")

    with tile.TileContext(nc) as tc:
        with (
            tc.tile_pool(name="wpool", bufs=1) as wpool,
            tc.tile_pool(name="xqk", bufs=2) as xqk,
            tc.tile_pool(name="xvs", bufs=2) as xvs,
            tc.tile_pool(name="big", bufs=1) as big,
            tc.tile_pool(name="ppool", bufs=8) as ppool,
            tc.tile_pool(name="small", bufs=4) as small,
            tc.tile_pool(name="osb", bufs=4) as osb,
            tc.tile_pool(name="ps", bufs=2, space="PSUM") as ps,
            tc.tile_pool(name="po", bufs=2, space="PSUM") as po,
            tc.tile_pool(name="psU", bufs=2, space="PSUM") as psU,
        ):
            # ---- resident weights / constants ----
            wq_t = wpool.tile([128, KT, GD], f16, tag="wq")
            wk_t = wpool.tile([128, KT, GD], f16, tag="wk")
            wv_t = wpool.tile([128, KT, GD], bf16, tag="wv")
            wo_t = wpool.tile([128, 2, D], bf16, tag="wo")
            bq_t = wpool.tile([128, 2, 1], f32, tag="bq")
            bk_t = wpool.tile([128, 2, 1], f32, tag="bk")
            sel0 = wpool.tile([1, 128], f32r, tag="sel0")
            sel1 = wpool.tile([1, 128], f32r, tag="sel1")
            sel0f = wpool.tile([1, 128], f32, tag="sel0f")
            sel1f = wpool.tile([1, 128], f32, tag="sel1f")
            warm_sink = wpool.tile([1, 16], f32, tag="wsink")

            # selector rows first (tiny) so warmup matmuls start ~t=0
            nc.sync.dma_start(out=sel0, in_=selg[0:1, :])
            nc.sync.dma_start(out=sel1, in_=selg[1:2, :])
            nc.scalar.dma_start(out=sel0f, in_=selg2[0:1, :])
            nc.scalar.dma_start(out=sel1f, in_=selg2[1:2, :])

            # ---- PE warmup: ~3.5us of junk outer products so the HAM
            # un-throttles before the first projection matmul ----
            wpsum = po.tile([128, 128], f32, tag="po", name="warm")
            for i in range(20):
                nc.tensor.matmul(
                    wpsum,
                    sel0,
                    sel0,
                    start=(i == 0),
                    stop=(i == 19),
                )
            nc.vector.tensor_copy(warm_sink, wpsum[0:1, 0:16])

            # ---- input streams ----
            # sync ring: wq, xq(m0), wk, xk(m0), then xq/xk m1..3
            # scalar ring: bq, bk, wv, xv(m0), wo, xv m1..3 (+ outputs later)
            nc.sync.dma_start(out=wq_t[:, 0:4, :], in_=xq_like_w(wqg)[:, 0:4, :])
            nc.scalar.dma_start(out=bq_t, in_=bqg[:].rearrange("t p o -> p t o"))
            nc.scalar.dma_start(out=bk_t, in_=bkg[:].rearrange("t p o -> p t o"))

            xq_ts, xk_ts, xv_ts = [], [], []
            xq_r = xq[:].rearrange("(kt p) s -> p kt s", p=128)
            xk_r = xk[:].rearrange("(kt p) s -> p kt s", p=128)
            xv_r = xv[:].rearrange("(kt p) s -> p kt s", p=128)

            def load_m(m):
                ms = slice(m * 512, (m + 1) * 512)
                xqt = xqk.tile([128, KT, 512], f16, tag="xq", name="xqt")
                xkt = xqk.tile([128, KT, 512], f16, tag="xk", name="xkt")
                xvt = xvs.tile([128, KT, 512], bf16, tag="xv", name="xvt")
                if m == 0:
                    # need-ordered halves so kk 0-3 matmuls start early
                    nc.sync.dma_start(out=xqt[:, 0:4, :], in_=xq_r[:, 0:4, ms])
                    nc.sync.dma_start(out=wq_t[:, 4:KT, :], in_=xq_like_w(wqg)[:, 4:KT, :])
                    nc.sync.dma_start(out=xqt[:, 4:KT, :], in_=xq_r[:, 4:KT, ms])
                    nc.sync.dma_start(out=wk_t[:, 0:4, :], in_=xq_like_w(wkg)[:, 0:4, :])
                else:
                    nc.sync.dma_start(out=xqt, in_=xq_r[:, :, ms])
                if m == 0:
                    nc.sync.dma_start(out=xkt[:, 0:4, :], in_=xk_r[:, 0:4, ms])
                    nc.sync.dma_start(out=wk_t[:, 4:KT, :], in_=xq_like_w(wkg)[:, 4:KT, :])
                    nc.sync.dma_start(out=xkt[:, 4:KT, :], in_=xk_r[:, 4:KT, ms])
                else:
                    nc.sync.dma_start(out=xkt, in_=xk_r[:, :, ms])
                if m == 0:
                    nc.sync.dma_start(out=wv_t, in_=xq_like_w(wvg))
                nc.sync.dma_start(out=xvt, in_=xv_r[:, :, ms])
                if m == 0:
                    nc.sync.dma_start(
                        out=wo_t, in_=wog[:].rearrange("(t p) n -> p t n", p=128)
                    )
                xq_ts.append(xqt)
                xk_ts.append(xkt)
                xv_ts.append(xvt)

            for m in range(MS):
                load_m(m)

            # ---- persistent activations ----
            qht = big.tile([128, 2, S], f16, tag="qht")
            kht = big.tile([128, 2, S], f16, tag="kht")
            vh = big.tile([128, JT, HG, DH + 1], bf16, tag="vh")
            ct = big.tile([128, 2, S], bf16, tag="ct")
            vh_ones_stage = wpool.tile([128, JT, HG, 1], f32, tag="vh_ones_st")
            nc.vector.memset(vh_ones_stage, 1.0)
            nc.scalar.activation(
                out=vh[:, :, :, DH : DH + 1], in_=vh_ones_stage, func=Act.Copy
            )

            # ---- projection chunk emitters (consumed as fillers) ----
            def qk_chunks(m):
                """QK projection chunk callables for m-slice m."""
                ms = slice(m * 512, (m + 1) * 512)
                qk_chunks.psums = {}

                def qk_chunk(xts, w_t, b_t, dst, n, kks):
                    def emit():
                        if kks[0] == 0:
                            psum = po.tile([128, 512], f32, tag="po", name="qkp")
                            qk_chunks.psums[(id(xts), n)] = psum
                        psum = qk_chunks.psums[(id(xts), n)]
                        for kk in kks:
                            nc.tensor.matmul(
                                psum,
                                w_t[:, kk, n * 128 : (n + 1) * 128],
                                xts[:, kk, :],
                                start=(kk == 0),
                                stop=(kk == KT - 1),
                            )
                        if kks[-1] == KT - 1:
                            nc.vector.tensor_scalar_add(
                                dst[:, n, ms], psum, b_t[:, n, :]
                            )

                    return emit

                out = []
                splits = (
                    [list(range(0, 4)), list(range(4, KT))]
                    if m == 0
                    else [list(range(KT))]
                )
                for n in range(2):
                    for kks in splits:
                        out.append(qk_chunk(xq_ts[m], wq_t, bq_t, qht, n, kks))
                for n in range(2):
                    for kks in splits:
                        out.append(qk_chunk(xk_ts[m], wk_t, bk_t, kht, n, kks))
                return out

            def v_chunks(m):
                def v_chunk(jj):
                    def emit():
                        j = m * 4 + jj
                        psum = po.tile([128, GD], f32, tag="po", name="vps")
                        for kk in range(KT):
                            nc.tensor.matmul(
                                psum,
                                xv_ts[m][:, kk, jj * 128 : (jj + 1) * 128],
                                wv_t[:, kk, :],
                                start=(kk == 0),
                                stop=(kk == KT - 1),
                            )
                        nc.vector.tensor_copy(
                            vh[:, j, :, 0:DH],
                            psum[:].rearrange("p (h d) -> p h d", h=HG),
                        )

                    return emit

                return [v_chunk(jj) for jj in range(4)]

            # ---- attention ----
            recips = {}

            def emit_attention_pair(IS, hp, filler=None):
                """Scores+exp+attnV pipeline for head-pair hp of i-slice IS.
                filler() is called between units to interleave other PE work."""
                i0 = IS * 512
                n_j = (IS + 1) * 4
                nt = hp
                u_psums = [
                    psU.tile([128, 512], f32, tag="u", name=f"u{e}")
                    for e in range(2)
                ]
                n_full = n_j - 4
                units = []
                for Jg in range(n_full // 2):
                    units.append(("full", Jg))
                for J in range(n_full, n_j):
                    units.append(("diag", J))
                pts = {}
                s_psums = {}

                def emit_scores(u):
                    kind, idx = u
                    if kind == "full":
                        for e in range(2):
                            lo = 64 * e
                            s_psum = ps.tile([128, 2, 512], f32, tag="ps")
                            for half in range(2):
                                J = 2 * idx + half
                                nc.tensor.matmul(
                                    s_psum[:, half, :],
                                    kht[lo : lo + DH, nt, J * 128 : (J + 1) * 128],
                                    qht[lo : lo + DH, nt, i0 : i0 + 512],
                                    start=True,
                                    stop=True,
                                )
                            s_psums[(e, u)] = s_psum
                    else:
                        J = idx
                        r = J * 128 - i0
                        s_psum = ps.tile([128, 2, 512], f32, tag="ps", name="sd")
                        for e in range(2):
                            lo = 64 * e
                            nc.tensor.matmul(
                                s_psum[:, e, 0 : 512 - r],
                                kht[lo : lo + DH, nt, J * 128 : (J + 1) * 128],
                                qht[lo : lo + DH, nt, i0 + r : i0 + 512],
                                start=True,
                                stop=True,
                            )
                        s_psums[(0, u)] = s_psum

                def emit_exp_mask(u):
                    kind, idx = u
                    if kind == "full":
                        for e in range(2):
                            pt = ppool.tile([128, 2, 512], bf16, tag="pt")
                            nc.scalar.activation(
                                out=pt, in_=s_psums[(e, u)], func=Act.Exp
                            )
                            pts[(e, u)] = pt
                    else:
                        r = idx * 128 - i0
                        w = 512 - r
                        pt = ppool.tile([128, 2, 512], bf16, tag="pt", name="ptd")
                        nc.scalar.activation(
                            out=pt[:, :, 0:w],
                            in_=s_psums[(0, u)][:, :, 0:w],
                            func=Act.Exp,
                        )
                        nc.gpsimd.affine_select(
                            out=pt[:, :, 0:w],
                            in_=pt[:, :, 0:w],
                            compare_op=Alu.is_ge,
                            fill=0.0,
                            base=0,
                            pattern=[[0, 2], [1, w]],
                            channel_multiplier=-1,
                        )
                        pts[(0, u)] = pt

                def emit_attnv(u):
                    kind, idx = u
                    for e in range(2):
                        if kind == "full":
                            for half in range(2):
                                J = 2 * idx + half
                                nc.tensor.matmul(
                                    u_psums[e][0 : DH + 1, :],
                                    vh[:, J, 2 * hp + e, :],
                                    pts[(e, u)][:, half, :],
                                    start=(J == 0),
                                    stop=False,
                                )
                        else:
                            J = idx
                            r = J * 128 - i0
                            nc.tensor.matmul(
                                u_psums[e][0 : DH + 1, r:512],
                                vh[:, J, 2 * hp + e, :],
                                pts[(0, u)][:, e, 0 : 512 - r],
                                start=(J == 0),
                                stop=(J == n_j - 1),
                            )

                emit_scores(units[0])
                emit_exp_mask(units[0])
                for ui in range(1, len(units)):
                    emit_scores(units[ui])
                    emit_exp_mask(units[ui])
                    emit_attnv(units[ui - 1])
                    if filler is not None:
                        filler()
                emit_attnv(units[-1])

                # evacuate U banks: ct rows out, fast-approx reciprocal of the
                # denominator row staged via SBUF partition 0 (the custom
                # recip ucode ignores partition offsets on PSUM reads —
                # HW-probed), then an f32 -> f32r convert for the selector
                # matmul.  In the tail (last pair of the kernel) the ct
                # copies go on the idle ACT engine and the f32r converts are
                # skipped (the tail selector matmul runs in fp32), shortening
                # the serial DVE chain that gates the final output projection.
                tail = IS == IST - 1 and hp == HG // 2 - 1
                for e in range(2):
                    rden = small.tile([1, 512], f32, tag="rden", name=f"rd{e}")
                    nc.vector.tensor_copy(rden, u_psums[e][DH : DH + 1, :])
                    recips[(IS, hp, e, "den")] = rden
                for e in range(2):
                    lo = 64 * e
                    if tail:
                        nc.scalar.activation(
                            out=ct[lo : lo + DH, nt, i0 : i0 + 512],
                            in_=u_psums[e][0:DH, :],
                            func=Act.Copy,
                        )
                    else:
                        nc.vector.tensor_copy(
                            ct[lo : lo + DH, nt, i0 : i0 + 512], u_psums[e][0:DH, :]
                        )
                    rcf = small.tile([1, 512], f32, tag="rcf", name=f"rcf{e}")
                    nc.vector.reciprocal_approx_fast(
                        out=rcf, in_=recips[(IS, hp, e, "den")]
                    )
                    if tail:
                        recips[(IS, hp, e)] = rcf
                    else:
                        rc = small.tile([1, 512], f32r, tag="rc", name=f"rc{e}")
                        nc.vector.tensor_copy(rc, rcf)
                        recips[(IS, hp, e)] = rc

            def emit_normalize_hp(IS, hp, tail=False):
                i0 = IS * 512
                if tail:
                    bc_psum = ps.tile([128, 2, 512], f32, tag="ps", name="bcp")[:, 0, :]
                    sels = ((0, sel0f), (1, sel1f))
                else:
                    bc_psum = po.tile([128, 512], f32, tag="po", name="bcp")
                    sels = ((0, sel0), (1, sel1))
                for e, sel in sels:
                    nc.tensor.matmul(
                        bc_psum,
                        sel,
                        recips[(IS, hp, e)],
                        start=(e == 0),
                        stop=(e == 1),
                    )
                nc.vector.tensor_mul(
                    ct[:, hp, i0 : i0 + 512],
                    bc_psum,
                    ct[:, hp, i0 : i0 + 512],
                )

            def emit_normalize(IS):
                for hp in range(HG // 2):
                    emit_normalize_hp(IS, hp)

            def outproj_its(IS):
                def mk(it):
                    def emit():
                        r0 = IS * 512 + it * 128
                        out_sb = osb.tile([128, D], bf16, tag="out")
                        for nn in range(2):
                            o_psum = po.tile([128, 512], f32, tag="po")
                            for t in range(2):
                                nc.tensor.matmul(
                                    o_psum,
                                    ct[:, t, r0 : r0 + 128],
                                    wo_t[:, t, nn * 512 : (nn + 1) * 512],
                                    start=(t == 0),
                                    stop=(t == 1),
                                )
                            nc.vector.tensor_copy(
                                out_sb[:, nn * 512 : (nn + 1) * 512], o_psum
                            )
                        nc.gpsimd.dma_start(out=outp[r0 : r0 + 128, :], in_=out_sb)

                    return emit

                return [mk(it) for it in range(4)]

            # ---- main schedule ----
            # Per-block PE/ACT balance: exp (ACT) grows with the i-slice
            # (causal triangle) while projection fillers shrink, so outproj
            # is deferred ~2 slices and m3's V chunks land in block 3 to
            # feed the PE there.  Normalize stays 1 slice behind attention.
            #   B0: att(0) + proj(m1)
            #   B1: att(1) + proj(m2)                 [norm(0) between pairs]
            #   B2: att(2) + proj(m3) QK + outproj(0) [norm(1)]
            #   B3: att(3) + proj(m3) V + outproj(1,2)[norm(2)]
            #   tail: norm(3) + outproj(3)
            for emit in qk_chunks(0) + v_chunks(0):
                emit()

            blocks = [
                (qk_chunks(1) + v_chunks(1), [], None),
                (qk_chunks(2) + v_chunks(2), [], 0),
                (qk_chunks(3) + outproj_its(0), [], 1),
                (v_chunks(3) + outproj_its(1), outproj_its(2), 2),
            ]
            for IS, (fill0, fill1, norm_is) in enumerate(blocks):
                pending = list(fill0)
                pending.reverse()

                def filler():
                    if pending:
                        pending.pop()()

                emit_attention_pair(IS, 0, filler=filler)
                while pending:
                    pending.pop()()
                if norm_is is not None:
                    emit_normalize(norm_is)
                pending = list(fill1)
                pending.reverse()
                if IS == IST - 1:
                    # normalize pair0 of the last slice early (its recips
                    # complete during pair1) so the tail only waits on pair1
                    pending.append(lambda: emit_normalize_hp(IST - 1, 0))
                emit_attention_pair(IS, 1, filler=filler)
                while pending:
                    pending.pop()()
            # tail: prefill the hp0-contraction half of the first two
            # output-projection tiles while the hp1 normalize chain drains
            tail_psums = {}
            for it in range(1):
                r0 = (IST - 1) * 512 + it * 128
                for nn in range(2):
                    o_psum = po.tile([128, 512], f32, tag="po", name="tp")
                    nc.tensor.matmul(
                        o_psum,
                        ct[:, 0, r0 : r0 + 128],
                        wo_t[:, 0, nn * 512 : (nn + 1) * 512],
                        start=True,
                        stop=False,
                    )
                    tail_psums[(it, nn)] = o_psum
            emit_normalize_hp(IST - 1, 1, tail=True)
            for it in range(4):
                r0 = (IST - 1) * 512 + it * 128
                out_sb = osb.tile([128, D], bf16, tag="out", name="osb_t")
                for nn in range(2):
                    if (it, nn) in tail_psums:
                        o_psum = tail_psums[(it, nn)]
                        nc.tensor.matmul(
                            o_psum,
                            ct[:, 1, r0 : r0 + 128],
                            wo_t[:, 1, nn * 512 : (nn + 1) * 512],
                            start=False,
                            stop=True,
                        )
                    else:
                        o_psum = po.tile([128, 512], f32, tag="po", name="tp2")
                        for t in range(2):
                            nc.tensor.matmul(
                                o_psum,
                                ct[:, t, r0 : r0 + 128],
                                wo_t[:, t, nn * 512 : (nn + 1) * 512],
                                start=(t == 0),
                                stop=(t == 1),
                            )
                    nc.vector.tensor_copy(out_sb[:, nn * 512 : (nn + 1) * 512], o_psum)
                nc.gpsimd.dma_start(out=outp[r0 : r0 + 128, :], in_=out_sb)

    nc.compile()
    return nc


def xq_like_w(w):
    return w[:].rearrange("(kt p) n -> p kt n", p=128)


def _get_nc():
    global _cached
    if _cached is None:
        _cached = _build()
    return _cached


def _in_maps(q, k, v, wq, bq, wk, bk, wv, bv, wo, bo):
    import ml_dtypes

    bf = ml_dtypes.bfloat16
    maps = []
    for c in range(8):
        b, g = c // G, c % G
        cs = slice(g * GD, (g + 1) * GD)
        maps.append(
            {
                "xq": np.ascontiguousarray(q[b].T).astype(np.float16),
                "xk": np.ascontiguousarray(k[b].T).astype(np.float16),
                "xv": np.ascontiguousarray(v[b].T).astype(bf),
                "wqg": np.ascontiguousarray(wq[:, cs]).astype(np.float16),
                "wkg": np.ascontiguousarray(wk[:, cs]).astype(np.float16),
                "wvg": np.ascontiguousarray(wv[:, cs]).astype(bf),
                "wog": np.ascontiguousarray(wo[cs, :]).astype(bf),
                "bqg": np.ascontiguousarray(bq[cs]).reshape(2, 128, 1),
                "bkg": np.ascontiguousarray(bk[cs]).reshape(2, 128, 1),
                "selg": _SEL,
                "selg2": _SEL,
            }
        )
    return maps


def run(inputs, trace=False, trace_kwargs=None):
    from concourse.bass_utils import run_bass_kernel_spmd

    nc = _get_nc()
    maps = _in_maps(**inputs)
    res = run_bass_kernel_spmd(
        nc, maps, list(range(8)), trace=trace, **(trace_kwargs or {})
    )
    out = np.zeros((B, S, D), np.float32)
    for c in range(8):
        out[c // G] += res.results[c]["outp"].astype(np.float32)
    # exact bias fold: C = U/colsum + 1 (x) bv  =>  out += bv @ wo + bo
    out += inputs["bv"].astype(np.float32) @ inputs["wo"].astype(np.float32)
    out += inputs["bo"].astype(np.float32)
    return out.astype(np.float32), res


def kernel(**inputs) -> np.ndarray:
    out, _ = run(inputs)
    return out


# revision 14
# speedup vs baseline: 1.0296x; 1.0068x over previous
"""Multi-head attention (B=2, S=2048, D=1024, H=16, causal, unscaled scores)
on 8 Trainium2 NeuronCores.

Sharding: 2 batches x 4 head-groups (4 heads each). Core c handles batch
c//4, heads 4*(c%4) .. 4*(c%4)+3. Each core computes its group's QKV
projections, causal attention, and a partial output projection
(row-slice of wo); the host sums the 4 partials per batch (the
all-reduce) and adds the bias terms.

Precision: Q/K path (xq, xk, wq, wk, qht, kht) is fp16 (10-bit
mantissa; scores accumulate in fp32 PSUM so exp() arguments stay
accurate); the V path (xv, wv, vh, exp-probabilities, attention
output, wo, final output) is bf16.  Measured end-to-end rel err
~4.7e-3 against fp32 — well inside the 2e-2 gate — while halving HBM
traffic (14.7MB read/core) and DVE cost.

Schedule (PE-density-first — the kernel is TensorE-bound; the HAM
clock gate halves the PE clock after ~3.4us of idleness, so the
emission order keeps the PE stream dense):
  - 20 warmup outer-product matmuls at t~0 hold the HAM activity
    window busy during the DMA prologue so real matmuls start at
    2.4 GHz; m0 loads are split/need-ordered and the first QK
    projection chunks run on half the contraction while the rest
    streams in.
  - all input loads ride the sync HWDGE ring in need-order as one
    strided DMA per (tensor, m-slice); output stores ride the gpsimd
    SWDGE ring so they never contend with input streaming or the ACT
    engine (which is ~80% busy with exp in the late i-slices).
  - projection work is emitted in small chunks INSIDE the attention
    unit loop, balanced per block against the causal-triangle exp
    load: att(IS) overlaps proj(m=IS+1); outproj is deferred ~2
    slices and m3's V chunks land in the last block to feed the PE
    where exp dominates.
  - softmax normalization is deferred one slice: the denominator row
    (accumulated free by the ones-column of VH during attnV) is
    staged to SBUF partition 0 (reciprocal_approx_fast's custom ucode
    ignores partition offsets on PSUM reads — HW-probed), inverted
    with the ~5x-faster approx reciprocal, converted f32->f32r, and a
    K=1 selector matmul pair broadcasts both rows into a [128,512]
    bank for one full-width in-place multiply of ct.  The final
    pair's chain is split across DVE and ACT with an fp32 selector
    matmul to shorten the kernel tail.
  - bias terms bv/bo are folded in exactly on the host
    (C = U/colsum + 1*bv since softmax rows sum to 1).
"""

import numpy as np

D = 1024
S = 2048
NH = 16
DH = 64
B = 2
G = 4            # head-groups = cores per batch
HG = NH // G     # 4 heads per group
GD = HG * DH     # 256 columns per group
KT = D // 128    # 8 k-tiles
MS = S // 512    # 4 m-slices
JT = S // 128    # 16 j-tiles
IST = S // 512   # 4 i-slices

_cached = None

_SEL = np.zeros((2, 128), np.float32)
_SEL[0, 0:64] = 1.0
_SEL[1, 64:128] = 1.0


def _build():
    from concourse import bacc
    import concourse.mybir as mybir
    import concourse.tile as tile

    f32 = mybir.dt.float32
    f32r = mybir.dt.float32r
    f16 = mybir.dt.float16
    bf16 = mybir.dt.bfloat16
    Act = mybir.ActivationFunctionType
    Alu = mybir.AluOpType

    nc = bacc.Bacc(None, target_bir_lowering=False)
    xq = nc.dram_tensor("xq", [D, S], f16, kind="ExternalInput")
    xk = nc.dram_tensor("xk", [D, S], f16, kind="ExternalInput")
    xv = nc.dram_tensor("xv", [D, S], bf16, kind="ExternalInput")
    wqg = nc.dram_tensor("wqg", [D, GD], f16, kind="ExternalInput")
    wkg = nc.dram_tensor("wkg", [D, GD], f16, kind="ExternalInput")
    wvg = nc.dram_tensor("wvg", [D, GD], bf16, kind="ExternalInput")
    wog = nc.dram_tensor("wog", [GD, D], bf16, kind="ExternalInput")
    bqg = nc.dram_tensor("bqg", [2, 128, 1], f32, kind="ExternalInput")
    bkg = nc.dram_tensor("bkg", [2, 128, 1], f32, kind="ExternalInput")
    selg = nc.dram_tensor("selg", [2, 128], f32r, kind="ExternalInput")
    selg2 = nc.dram_tensor("selg2", [2, 128], f32, kind="ExternalInput")
    outp = nc.dram_tensor("outp", [S, D], bf16, kind="ExternalOutput")

    with tile.TileContext(nc) as tc:
        with (
            tc.tile_pool(name="wpool", bufs=1) as wpool,
            tc.tile_pool(name="xqk", bufs=2) as xqk,
            tc.tile_pool(name="xvs", bufs=3) as xvs,
            tc.tile_pool(name="big", bufs=1) as big,
            tc.tile_pool(name="ppool", bufs=10) as ppool,
            tc.tile_pool(name="small", bufs=6) as small,
            tc.tile_pool(name="osb", bufs=6) as osb,
            tc.tile_pool(name="ps", bufs=2, space="PSUM") as ps,
            tc.tile_pool(name="po", bufs=2, space="PSUM") as po,
            tc.tile_pool(name="psU", bufs=2, space="PSUM") as psU,
        ):
            # ---- resident weights / constants ----
            wq_t = wpool.tile([128, KT, GD], f16, tag="wq")
            wk_t = wpool.tile([128, KT, GD], f16, tag="wk")
            wv_t = wpool.tile([128, KT, GD], bf16, tag="wv")
            wo_t = wpool.tile([128, 2, D], bf16, tag="wo")
            bq_t = wpool.tile([128, 2, 1], f32, tag="bq")
            bk_t = wpool.tile([128, 2, 1], f32, tag="bk")
            sel0 = wpool.tile([1, 128], f32r, tag="sel0")
            sel1 = wpool.tile([1, 128], f32r, tag="sel1")
            sel0f = wpool.tile([1, 128], f32, tag="sel0f")
            sel1f = wpool.tile([1, 128], f32, tag="sel1f")
            warm_sink = wpool.tile([1, 16], f32, tag="wsink")

            # selector rows first (tiny) so warmup matmuls start ~t=0
            nc.sync.dma_start(out=sel0, in_=selg[0:1, :])
            nc.sync.dma_start(out=sel1, in_=selg[1:2, :])
            nc.scalar.dma_start(out=sel0f, in_=selg2[0:1, :])
            nc.scalar.dma_start(out=sel1f, in_=selg2[1:2, :])

            # ---- PE warmup: ~3.5us of junk outer products so the HAM
            # un-throttles before the first projection matmul ----
            wpsum = po.tile([128, 128], f32, tag="po", name="warm")
            for i in range(20):
                nc.tensor.matmul(
                    wpsum,
                    sel0,
                    sel0,
                    start=(i == 0),
                    stop=(i == 19),
                )
            nc.vector.tensor_copy(warm_sink, wpsum[0:1, 0:16])

            # ---- input streams ----
            # sync ring: wq, xq(m0), wk, xk(m0), then xq/xk m1..3
            # scalar ring: bq, bk, wv, xv(m0), wo, xv m1..3 (+ outputs later)
            nc.sync.dma_start(out=wq_t[:, 0:4, :], in_=xq_like_w(wqg)[:, 0:4, :])
            nc.scalar.dma_start(out=bq_t, in_=bqg[:].rearrange("t p o -> p t o"))
            nc.scalar.dma_start(out=bk_t, in_=bkg[:].rearrange("t p o -> p t o"))

            xq_ts, xk_ts, xv_ts = [], [], []
            xq_r = xq[:].rearrange("(kt p) s -> p kt s", p=128)
            xk_r = xk[:].rearrange("(kt p) s -> p kt s", p=128)
            xv_r = xv[:].rearrange("(kt p) s -> p kt s", p=128)

            def load_m(m):
                ms = slice(m * 512, (m + 1) * 512)
                xqt = xqk.tile([128, KT, 512], f16, tag="xq", name="xqt")
                xkt = xqk.tile([128, KT, 512], f16, tag="xk", name="xkt")
                xvt = xvs.tile([128, KT, 512], bf16, tag="xv", name="xvt")
                if m == 0:
                    # need-ordered halves so kk 0-3 matmuls start early
                    nc.sync.dma_start(out=xqt[:, 0:4, :], in_=xq_r[:, 0:4, ms])
                    nc.sync.dma_start(out=wq_t[:, 4:KT, :], in_=xq_like_w(wqg)[:, 4:KT, :])
                    nc.sync.dma_start(out=xqt[:, 4:KT, :], in_=xq_r[:, 4:KT, ms])
                    nc.sync.dma_start(out=wk_t[:, 0:4, :], in_=xq_like_w(wkg)[:, 0:4, :])
                else:
                    nc.sync.dma_start(out=xqt, in_=xq_r[:, :, ms])
                if m == 0:
                    nc.sync.dma_start(out=xkt[:, 0:4, :], in_=xk_r[:, 0:4, ms])
                    nc.sync.dma_start(out=wk_t[:, 4:KT, :], in_=xq_like_w(wkg)[:, 4:KT, :])
                    nc.sync.dma_start(out=xkt[:, 4:KT, :], in_=xk_r[:, 4:KT, ms])
                else:
                    nc.sync.dma_start(out=xkt, in_=xk_r[:, :, ms])
                if m == 0:
                    nc.sync.dma_start(out=wv_t, in_=xq_like_w(wvg))
                nc.sync.dma_start(out=xvt, in_=xv_r[:, :, ms])
                if m == 0:
                    nc.sync.dma_start(
                        out=wo_t, in_=wog[:].rearrange("(t p) n -> p t n", p=128)
                    )
                xq_ts.append(xqt)
                xk_ts.append(xkt)
                xv_ts.append(xvt)

            for m in range(MS):
                load_m(m)

            # ---- persistent activations ----
            qht = big.tile([128, 2, S], f16, tag="qht")
            kht = big.tile([128, 2, S], f16, tag="kht")
            vh = big.tile([128, JT, HG, DH + 1], bf16, tag="vh")
            ct = big.tile([128, 2, S], bf16, tag="ct")
            vh_ones_stage = wpool.tile([128, JT, HG, 1], f32, tag="vh_ones_st")
            nc.vector.memset(vh_ones_stage, 1.0)
            nc.scalar.activation(
                out=vh[:, :, :, DH : DH + 1], in_=vh_ones_stage, func=Act.Copy
            )

            # ---- projection chunk emitters (consumed as fillers) ----
            def qk_chunks(m):
                """QK projection chunk callables for m-slice m."""
                ms = slice(m * 512, (m + 1) * 512)
                qk_chunks.psums = {}

                def qk_chunk(xts, w_t, b_t, dst, n, kks):
                    def emit():
                        if kks[0] == 0:
                            psum = po.tile([128, 512], f32, tag="po", name="qkp")
                            qk_chunks.psums[(id(xts), n)] = psum
                        psum = qk_chunks.psums[(id(xts), n)]
                        for kk in kks:
                            nc.tensor.matmul(
                                psum,
                                w_t[:, kk, n * 128 : (n + 1) * 128],
                                xts[:, kk, :],
                                start=(kk == 0),
                                stop=(kk == KT - 1),
                            )
                        if kks[-1] == KT - 1:
                            nc.vector.tensor_scalar_add(
                                dst[:, n, ms], psum, b_t[:, n, :]
                            )

                    return emit

                out = []
                splits = (
                    [list(range(0, 4)), list(range(4, KT))]
                    if m == 0
                    else [list(range(KT))]
                )
                for n in range(2):
                    for kks in splits:
                        out.append(qk_chunk(xq_ts[m], wq_t, bq_t, qht, n, kks))
                for n in range(2):
                    for kks in splits:
                        out.append(qk_chunk(xk_ts[m], wk_t, bk_t, kht, n, kks))
                return out

            def v_chunks(m):
                def v_chunk(jj):
                    def emit():
                        j = m * 4 + jj
                        psum = po.tile([128, GD], f32, tag="po", name="vps")
                        for kk in range(KT):
                            nc.tensor.matmul(
                                psum,
                                xv_ts[m][:, kk, jj * 128 : (jj + 1) * 128],
                                wv_t[:, kk, :],
                                start=(kk == 0),
                                stop=(kk == KT - 1),
                            )
                        nc.vector.tensor_copy(
                            vh[:, j, :, 0:DH],
                            psum[:].rearrange("p (h d) -> p h d", h=HG),
                        )

                    return emit

                return [v_chunk(jj) for jj in range(4)]

            # ---- attention ----
            recips = {}

            def emit_attention_pair(IS, hp, filler=None):
                """Scores+exp+attnV pipeline for head-pair hp of i-slice IS.
                filler() is called between units to interleave other PE work."""
                i0 = IS * 512
                n_j = (IS + 1) * 4
                nt = hp
                u_psums = [
                    psU.tile([128, 512], f32, tag="u", name=f"u{e}")
                    for e in range(2)
                ]
                n_full = n_j - 4
                units = []
                for Jg in range(n_full // 2):
                    units.append(("full", Jg))
                for J in range(n_full, n_j):
                    units.append(("diag", J))
                pts = {}
                s_psums = {}

                def emit_scores(u):
                    kind, idx = u
                    if kind == "full":
                        for e in range(2):
                            lo = 64 * e
                            s_psum = ps.tile([128, 2, 512], f32, tag="ps")
                            for half in range(2):
                                J = 2 * idx + half
                                nc.tensor.matmul(
                                    s_psum[:, half, :],
                                    kht[lo : lo + DH, nt, J * 128 : (J + 1) * 128],
                                    qht[lo : lo + DH, nt, i0 : i0 + 512],
                                    start=True,
                                    stop=True,
                                )
                            s_psums[(e, u)] = s_psum
                    else:
                        J = idx
                        r = J * 128 - i0
                        s_psum = ps.tile([128, 2, 512], f32, tag="ps", name="sd")
                        for e in range(2):
                            lo = 64 * e
                            nc.tensor.matmul(
                                s_psum[:, e, 0 : 512 - r],
                                kht[lo : lo + DH, nt, J * 128 : (J + 1) * 128],
                                qht[lo : lo + DH, nt, i0 + r : i0 + 512],
                                start=True,
                                stop=True,
                            )
                        s_psums[(0, u)] = s_psum

                def emit_exp_mask(u):
                    kind, idx = u
                    if kind == "full":
                        for e in range(2):
                            pt = ppool.tile([128, 2, 512], bf16, tag="pt")
                            nc.scalar.activation(
                                out=pt, in_=s_psums[(e, u)], func=Act.Exp
                            )
                            pts[(e, u)] = pt
                    else:
                        r = idx * 128 - i0
                        w = 512 - r
                        pt = ppool.tile([128, 2, 512], bf16, tag="pt", name="ptd")
                        nc.scalar.activation(
                            out=pt[:, :, 0:w],
                            in_=s_psums[(0, u)][:, :, 0:w],
                            func=Act.Exp,
                        )
                        nc.gpsimd.affine_select(
                            out=pt[:, :, 0:w],
                            in_=pt[:, :, 0:w],
                            compare_op=Alu.is_ge,
                            fill=0.0,
                            base=0,
                            pattern=[[0, 2], [1, w]],
                            channel_multiplier=-1,
                        )
                        pts[(0, u)] = pt

                def emit_attnv(u):
                    kind, idx = u
                    for e in range(2):
                        if kind == "full":
                            for half in range(2):
                                J = 2 * idx + half
                                nc.tensor.matmul(
                                    u_psums[e][0 : DH + 1, :],
                                    vh[:, J, 2 * hp + e, :],
                                    pts[(e, u)][:, half, :],
                                    start=(J == 0),
                                    stop=False,
                                )
                        else:
                            J = idx
                            r = J * 128 - i0
                            nc.tensor.matmul(
                                u_psums[e][0 : DH + 1, r:512],
                                vh[:, J, 2 * hp + e, :],
                                pts[(0, u)][:, e, 0 : 512 - r],
                                start=(J == 0),
                                stop=(J == n_j - 1),
                            )

                emit_scores(units[0])
                emit_exp_mask(units[0])
                for ui in range(1, len(units)):
                    emit_scores(units[ui])
                    emit_exp_mask(units[ui])
                    emit_attnv(units[ui - 1])
                    if filler is not None:
                        filler()
                emit_attnv(units[-1])

                # evacuate U banks: ct rows out, fast-approx reciprocal of the
                # denominator row staged via SBUF partition 0 (the custom
                # recip ucode ignores partition offsets on PSUM reads —
                # HW-probed), then an f32 -> f32r convert for the selector
                # matmul.  In the tail (last pair of the kernel) the ct
                # copies go on the idle ACT engine and the f32r converts are
                # skipped (the tail selector matmul runs in fp32), shortening
                # the serial DVE chain that gates the final output projection.
                tail = IS == IST - 1 and hp == HG // 2 - 1
                for e in range(2):
                    rden = small.tile([1, 512], f32, tag="rden", name=f"rd{e}")
                    nc.vector.tensor_copy(rden, u_psums[e][DH : DH + 1, :])
                    recips[(IS, hp, e, "den")] = rden
                for e in range(2):
                    lo = 64 * e
                    if tail:
                        nc.scalar.activation(
                            out=ct[lo : lo + DH, nt, i0 : i0 + 512],
                            in_=u_psums[e][0:DH, :],
                            func=Act.Copy,
                        )
                    else:
                        nc.vector.tensor_copy(
                            ct[lo : lo + DH, nt, i0 : i0 + 512], u_psums[e][0:DH, :]
                        )
                    rcf = small.tile([1, 512], f32, tag="rcf", name=f"rcf{e}")
                    nc.vector.reciprocal_approx_fast(
                        out=rcf, in_=recips[(IS, hp, e, "den")]
                    )
                    if tail:
                        recips[(IS, hp, e)] = rcf
                    else:
                        rc = small.tile([1, 512], f32r, tag="rc", name=f"rc{e}")
                        nc.vector.tensor_copy(rc, rcf)
                        recips[(IS, hp, e)] = rc

            def emit_normalize_hp(IS, hp, tail=False):
                i0 = IS * 512
                if tail:
                    bc_psum = ps.tile([128, 2, 512], f32, tag="ps", name="bcp")[:, 0, :]
                    sels = ((0, sel0f), (1, sel1f))
                else:
                    bc_psum = po.tile([128, 512], f32, tag="po", name="bcp")
                    sels = ((0, sel0), (1, sel1))
                for e, sel in sels:
                    nc.tensor.matmul(
                        bc_psum,
                        sel,
                        recips[(IS, hp, e)],
                        start=(e == 0),
                        stop=(e == 1),
                    )
                nc.vector.tensor_mul(
                    ct[:, hp, i0 : i0 + 512],
                    bc_psum,
                    ct[:, hp, i0 : i0 + 512],
                )

            def emit_normalize(IS):
                for hp in range(HG // 2):
                    emit_normalize_hp(IS, hp)

            def outproj_its(IS):
                def mk(it):
                    def emit():
                        r0 = IS * 512 + it * 128
                        out_sb = osb.tile([128, D], bf16, tag="out")
                        for nn in range(2):
                            o_psum = po.tile([128, 512], f32, tag="po")
                            for t in range(2):
                                nc.tensor.matmul(
                                    o_psum,
                                    ct[:, t, r0 : r0 + 128],
                                    wo_t[:, t, nn * 512 : (nn + 1) * 512],
                                    start=(t == 0),
                                    stop=(t == 1),
                                )
                            nc.vector.tensor_copy(
                                out_sb[:, nn * 512 : (nn + 1) * 512], o_psum
                            )
                        nc.gpsimd.dma_start(out=outp[r0 : r0 + 128, :], in_=out_sb)

                    return emit

                return [mk(it) for it in range(4)]

            # ---- main schedule ----
            # Per-block PE/ACT balance: exp (ACT) grows with the i-slice
            # (causal triangle) while projection fillers shrink, so outproj
            # is deferred ~2 slices and m3's V chunks land in block 3 to
            # feed the PE there.  Normalize stays 1 slice behind attention.
            #   B0: att(0) + proj(m1)
            #   B1: att(1) + proj(m2)                 [norm(0) between pairs]
            #   B2: att(2) + proj(m3) QK + outproj(0) [norm(1)]
            #   B3: att(3) + proj(m3) V + outproj(1,2)[norm(2)]
            #   tail: norm(3) + outproj(3)
            for emit in qk_chunks(0) + v_chunks(0):
                emit()

            blocks = [
                (qk_chunks(1) + v_chunks(1), [], None),
                (qk_chunks(2) + v_chunks(2), [], 0),
                (qk_chunks(3) + outproj_its(0), [], 1),
                (v_chunks(3) + outproj_its(1), outproj_its(2), 2),
            ]
            for IS, (fill0, fill1, norm_is) in enumerate(blocks):
                pending = list(fill0)
                pending.reverse()

                def filler():
                    if pending:
                        pending.pop()()

                emit_attention_pair(IS, 0, filler=filler)
                while pending:
                    pending.pop()()
                if norm_is is not None:
                    emit_normalize(norm_is)
                pending = list(fill1)
                pending.reverse()
                if IS == IST - 1:
                    # normalize pair0 of the last slice early (its recips
                    # complete during pair1) so the tail only waits on pair1
                    pending.append(lambda: emit_normalize_hp(IST - 1, 0))
                emit_attention_pair(IS, 1, filler=filler)
                while pending:
                    pending.pop()()
            # tail: prefill the hp0-contraction half of the first two
            # output-projection tiles while the hp1 normalize chain drains
            tail_psums = {}
            for it in range(1):
                r0 = (IST - 1) * 512 + it * 128
                for nn in range(2):
                    o_psum = po.tile([128, 512], f32, tag="po", name="tp")
                    nc.tensor.matmul(
                        o_psum,
                        ct[:, 0, r0 : r0 + 128],
                        wo_t[:, 0, nn * 512 : (nn + 1) * 512],
                        start=True,
                        stop=False,
                    )
                    tail_psums[(it, nn)] = o_psum
            emit_normalize_hp(IST - 1, 1, tail=True)
            for it in range(4):
                r0 = (IST - 1) * 512 + it * 128
                out_sb = osb.tile([128, D], bf16, tag="out", name="osb_t")
                for nn in range(2):
                    if (it, nn) in tail_psums:
                        o_psum = tail_psums[(it, nn)]
                        nc.tensor.matmul(
                            o_psum,
                            ct[:, 1, r0 : r0 + 128],
                            wo_t[:, 1, nn * 512 : (nn + 1) * 512],
                            start=False,
                            stop=True,
                        )
                    else:
                        o_psum = po.tile([128, 512], f32, tag="po", name="tp2")
                        for t in range(2):
                            nc.tensor.matmul(
                                o_psum,
                                ct[:, t, r0 : r0 + 128],
                                wo_t[:, t, nn * 512 : (nn + 1) * 512],
                                start=(t == 0),
                                stop=(t == 1),
                            )
                    nc.vector.tensor_copy(out_sb[:, nn * 512 : (nn + 1) * 512], o_psum)
                nc.gpsimd.dma_start(out=outp[r0 : r0 + 128, :], in_=out_sb)

    nc.compile()
    return nc


def xq_like_w(w):
    return w[:].rearrange("(kt p) n -> p kt n", p=128)


def _get_nc():
    global _cached
    if _cached is None:
        _cached = _build()
    return _cached


def _in_maps(q, k, v, wq, bq, wk, bk, wv, bv, wo, bo):
    import ml_dtypes

    bf = ml_dtypes.bfloat16
    maps = []
    for c in range(8):
        b, g = c // G, c % G
        cs = slice(g * GD, (g + 1) * GD)
        maps.append(
            {
                "xq": np.ascontiguousarray(q[b].T).astype(np.float16),
                "xk": np.ascontiguousarray(k[b].T).astype(np.float16),
                "xv": np.ascontiguousarray(v[b].T).astype(bf),
                "wqg": np.ascontiguousarray(wq[:, cs]).astype(np.float16),
                "wkg": np.ascontiguousarray(wk[:, cs]).astype(np.float16),
                "wvg": np.ascontiguousarray(wv[:, cs]).astype(bf),
                "wog": np.ascontiguousarray(wo[cs, :]).astype(bf),
                "bqg": np.ascontiguousarray(bq[cs]).reshape(2, 128, 1),
                "bkg": np.ascontiguousarray(bk[cs]).reshape(2, 128, 1),
                "selg": _SEL,
                "selg2": _SEL,
            }
        )
    return maps


def run(inputs, trace=False, trace_kwargs=None):
    from concourse.bass_utils import run_bass_kernel_spmd

    nc = _get_nc()
    maps = _in_maps(**inputs)
    res = run_bass_kernel_spmd(
        nc, maps, list(range(8)), trace=trace, **(trace_kwargs or {})
    )
    out = np.zeros((B, S, D), np.float32)
    for c in range(8):
        out[c // G] += res.results[c]["outp"].astype(np.float32)
    # exact bias fold: C = U/colsum + 1 (x) bv  =>  out += bv @ wo + bo
    out += inputs["bv"].astype(np.float32) @ inputs["wo"].astype(np.float32)
    out += inputs["bo"].astype(np.float32)
    return out.astype(np.float32), res


def kernel(**inputs) -> np.ndarray:
    out, _ = run(inputs)
    return out
